# revision 1
# baseline (speedup 1.0000x reference)
"""Trainium2 Bass kernel for nn_Local_EncoderLayer (local+global sparse attention encoder).

Sharding: data-parallel over batch B=8 across 8 cores (one batch per core).
Both attention stages and the local/global regroup are batch-internal, so
there is no cross-core communication.

Per-core layout: activations are kept transposed, X^T [D=1024, T=2048], as
float32r (TF32) resident in SBUF. Big GEMMs (QKV / proj / FFN) run in fp32r
at full PE rate; the tiny attention-internal matmuls (scores, attn@V) run in
bf16. The block-diagonal attention mask is folded into the score matmul as
extra contraction rows (rank trick, additive -50 off-block). Softmax uses
ACT exp with accum_out; P is transposed on the PE and the reference's
"attn@V - V" is computed as (P^T - I) via one fused DVE op. LayerNorm's
cross-partition mean/var use ones-vector matmuls plus a broadcast matmul.
The local->global regroup is pure access-pattern (strided token views).
"""
import functools
import numpy as np
import ml_dtypes

import concourse.bass as bass
import concourse.tile as tile
from concourse import bacc, mybir
from concourse.bass import ds
from concourse.bass_utils import run_bass_kernel_spmd

B, L, D, H, DK, DV, DI, NL = 8, 2048, 1024, 16, 64, 64, 4096, 32
EPS = 1e-5
P = 128
T = L                   # tokens per core
DC = D // P             # 8 d-chunks
FC = (H * DK) // P      # 8 head-pair chunks
TP = 1024               # tokens per pass
NPASS = T // TP
NTT = TP // 512         # 512-token tiles per pass
NG = TP // P            # groups (of 128 tokens) per pass
GSEQ = T // NL          # global-stage sequence length (64)
SPG = P // GSEQ         # sequences per group in global stage (2)
NDIC = DI // 512        # FFN inner chunks

F32 = mybir.dt.float32
F32R = mybir.dt.float32r
BF16 = mybir.dt.bfloat16
AF = mybir.ActivationFunctionType
ALU = mybir.AluOpType
MASK_C = float(np.sqrt(50.0))


def _din(nc, name, shape, dt=F32):
    return nc.dram_tensor(name, shape, dt, kind="ExternalInput").ap()


def _build_nc(repeat=1):
    nc = bacc.Bacc("TRN2", target_bir_lowering=False, debug=False, num_devices=8)
    x_in = _din(nc, "x", [T, D])
    y_out = nc.dram_tensor("y", [T, D], F32, kind="ExternalOutput").ap()

    W = {}
    for pfx in ("la", "sa"):
        W[pfx] = dict(
            wq=_din(nc, f"{pfx}_wq", [D, H * DK], F32R),
            wk=_din(nc, f"{pfx}_wk", [D, H * DK], F32R),
            wv=_din(nc, f"{pfx}_wv", [D, H * DV], F32R),
            pw=_din(nc, f"{pfx}_pw", [H * DV, D], F32R),
            pb=_din(nc, f"{pfx}_pb", [D]),
            g=_din(nc, f"{pfx}_g", [D]),
            b=_din(nc, f"{pfx}_b", [D]),
        )
    for pfx in ("lf", "pf"):
        W[pfx] = dict(
            w1=_din(nc, f"{pfx}_w1", [D, DI], F32R),
            b1=_din(nc, f"{pfx}_b1", [DI]),
            w2=_din(nc, f"{pfx}_w2", [DI, D], F32R),
            b2=_din(nc, f"{pfx}_b2", [D]),
            g=_din(nc, f"{pfx}_g", [D]),
            b=_din(nc, f"{pfx}_b", [D]),
        )
    id32 = _din(nc, "id32", [P, P], F32)
    id32r = _din(nc, "id32r", [P, P], F32R)
    idbf = _din(nc, "idbf", [P, P], BF16)
    mq_l = _din(nc, "mq_l", [NL // 8 + 1, P], BF16)   # 5 rows
    mk_l = _din(nc, "mk_l", [NL // 8 + 1, P], BF16)
    mq_g = _din(nc, "mq_g", [SPG + 1, P], BF16)
    mk_g = _din(nc, "mk_g", [SPG + 1, P], BF16)
    ones_col = _din(nc, "ones_col", [P, 1], F32R)
    eps_col = _din(nc, "eps_col", [P, 1], F32)
    invd_row = _din(nc, "invd_row", [1, P], F32R)

    with tile.TileContext(nc) as tc:
        for _rep in range(repeat):
            _body(nc, tc, x_in, y_out, W,
                  dict(id32=id32, id32r=id32r, idbf=idbf,
                       mq_l=mq_l, mk_l=mk_l, mq_g=mq_g, mk_g=mk_g,
                       ones_col=ones_col, invd_row=invd_row, eps_col=eps_col))
    nc.compile()
    return nc


def _body(nc, tc, x_in, y_out, W, consts):
    from contextlib import ExitStack
    ctx = ExitStack()
    with ctx:
        cp = ctx.enter_context(tc.tile_pool(name="const", bufs=1))
        xp = ctx.enter_context(tc.tile_pool(name="xres", bufs=1))
        wp = ctx.enter_context(tc.tile_pool(name="wstream", bufs=8))

        # ---- consts to SBUF
        def cload(name, shape, dt):
            t = cp.tile(shape, dt, tag=name, name=name)
            nc.sync.dma_start(t[:], consts[name])
            return t
        id32_t = cload("id32", [P, P], F32)
        id32r_t = cload("id32r", [P, P], F32R)
        idbf_t = cload("idbf", [P, P], BF16)
        mq_l_t = cload("mq_l", [5, P], BF16)
        mk_l_t = cload("mk_l", [5, P], BF16)
        mq_g_t = cload("mq_g", [SPG + 1, P], BF16)
        mk_g_t = cload("mk_g", [SPG + 1, P], BF16)
        ones_t = cload("ones_col", [P, 1], F32R)
        invd_t = cload("invd_row", [1, P], F32R)
        eps_t = cload("eps_col", [P, 1], F32)

        def vec_tile(ap, n, name):
            # [n] dram vector -> [P, n//P] sbuf tile (col c = chunk c)
            t = cp.tile([P, n // P], F32, tag=name, name=name)
            nc.sync.dma_start(t[:], ap.rearrange("(c p) -> p c", p=P))
            return t
        VT = {}
        for pfx in ("la", "sa"):
            VT[pfx] = dict(
                pb=vec_tile(W[pfx]["pb"], D, f"{pfx}_pb"),
                g=vec_tile(W[pfx]["g"], D, f"{pfx}_g"),
                b=vec_tile(W[pfx]["b"], D, f"{pfx}_b"),
            )
        for pfx in ("lf", "pf"):
            VT[pfx] = dict(
                b1=vec_tile(W[pfx]["b1"], DI, f"{pfx}_b1"),
                b2=vec_tile(W[pfx]["b2"], D, f"{pfx}_b2"),
                g=vec_tile(W[pfx]["g"], D, f"{pfx}_g"),
                b=vec_tile(W[pfx]["b"], D, f"{pfx}_b"),
            )

        # ---- resident X^T tiles
        X = [xp.tile([P, T], F32R, tag=f"x{dc}", name=f"x{dc}") for dc in range(DC)]

        def xcols(dc, glob, p, j0, n):
            """AP view of X[dc] columns for pass p, grouped-token range [j0, j0+n)."""
            if not glob:
                return X[dc][:, ds(TP * p + j0, n)]
            # grouped index j = GSEQ*(s - s0) + k ; token t = s + NL*k
            Xr = X[dc].rearrange("p (k s) -> p s k", s=NL)  # [P, 32, 64]
            s0 = (TP // GSEQ) * p + j0 // GSEQ
            return Xr[:, s0: s0 + n // GSEQ, :]

        def gv(ap, glob):
            """Reshape a contiguous [P, n] view to [P, n//GSEQ, GSEQ] to match strided views."""
            if not glob:
                return ap
            return ap.rearrange("p (a b) -> p a b", b=GSEQ)

        # ---- input transpose: x [T, D] -> X^T
        with (
            tc.tile_pool(name="io_sb", bufs=3) as iop,
            tc.tile_pool(name="io_ps", bufs=4, space="PSUM") as iops,
        ):
            for tg in range(T // P):
                nat = iop.tile([P, D], F32, tag="nat")
                nc.sync.dma_start(nat[:], x_in[ds(P * tg, P), :])
                for dc in range(DC):
                    tp = iops.tile([P, P], F32, tag="tp")
                    nc.tensor.transpose(tp[:], nat[:, ds(P * dc, P)], id32_t[:])
                    nc.scalar.activation(X[dc][:, ds(P * tg, P)], tp[:], AF.Copy)

        # ---- layernorm: in-place over X columns (Z already written into X)
        def layer_norm_writeback(g_t, b_t, glob, p, lnp, lnps):
            for tt in range(NTT):
                s1p = lnps.tile([1, 512], F32, tag="s1")
                sqp = lnps.tile([1, 512], F32, tag="sq")
                for dc in range(DC):
                    zsl = xcols(dc, glob, p, 512 * tt, 512)
                    sq_t = lnp.tile([P, 512], F32R, tag="sqt", bufs=1)
                    nc.vector.tensor_mul(gv(sq_t[:], glob), zsl, zsl)
                    nc.tensor.matmul(s1p[:], ones_t[:], zsl,
                                     start=(dc == 0), stop=(dc == DC - 1))
                    nc.tensor.matmul(sqp[:], ones_t[:], sq_t[:],
                                     start=(dc == 0), stop=(dc == DC - 1))
                s1s = lnp.tile([1, 512], F32R, tag="s1s", bufs=1)
                nc.vector.tensor_copy(s1s[:], s1p[:])
                sqs = lnp.tile([1, 512], F32R, tag="sqs", bufs=1)
                nc.vector.tensor_copy(sqs[:], sqp[:])
                mup = lnps.tile([P, 512], F32, tag="mub")
                nc.tensor.matmul(mup[:], invd_t[:], s1s[:], start=True, stop=True)
                sqb = lnps.tile([P, 512], F32, tag="sqb")
                nc.tensor.matmul(sqb[:], invd_t[:], sqs[:], start=True, stop=True)
                mu_sb = lnp.tile([P, 512], F32, tag="mu_sb", bufs=1)
                nc.vector.tensor_copy(mu_sb[:], mup[:])
                mu2 = lnp.tile([P, 512], F32, tag="mu2", bufs=1)
                nc.vector.tensor_mul(mu2[:], mu_sb[:], mu_sb[:])
                var = lnp.tile([P, 512], F32, tag="var", bufs=1)
                nc.vector.scalar_tensor_tensor(var[:], mu2[:], -1.0, sqb[:],
                                               op0=ALU.mult, op1=ALU.add)
                sig = lnp.tile([P, 512], F32, tag="sig", bufs=1)
                nc.scalar.activation(sig[:], var[:], AF.Sqrt, bias=eps_t[:])
                rsig = lnp.tile([P, 512], F32, tag="rsig", bufs=1)
                nc.vector.reciprocal(rsig[:], sig[:])
                for dc in range(DC):
                    zsl = xcols(dc, glob, p, 512 * tt, 512)
                    nc.vector.scalar_tensor_tensor(zsl, zsl, 0.0, gv(mu_sb[:], glob),
                                                   op0=ALU.add, op1=ALU.subtract)
                    nc.vector.scalar_tensor_tensor(zsl, zsl, g_t[:, dc:dc + 1],
                                                   gv(rsig[:], glob),
                                                   op0=ALU.mult, op1=ALU.mult)
                    nc.vector.tensor_scalar_add(zsl, zsl, b_t[:, dc:dc + 1])

        # ---- attention stage
        def attn_stage(pfx, glob):
            w = W[pfx]
            vt = VT[pfx]
            mq_t, mk_t = (mq_g_t, mk_g_t) if glob else (mq_l_t, mk_l_t)
            with (
                tc.tile_pool(name=f"{pfx}_buf", bufs=1) as bp,
                tc.tile_pool(name=f"{pfx}_small", bufs=1) as sp,
            ):
                for p in range(NPASS):
                    QT = [bp.tile([P, TP], BF16, tag=f"qt{fc}", name=f"qt{fc}") for fc in range(FC)]
                    KT = [bp.tile([P, TP], BF16, tag=f"kt{fc}", name=f"kt{fc}") for fc in range(FC)]
                    attT = [bp.tile([P, TP], F32R, tag=f"at{oc}", name=f"at{oc}") for oc in range(DC)]

                    # --- Q^T / K^T production (fp32r)
                    with tc.tile_pool(name="qk_ps", bufs=1, space="PSUM") as qkps:
                        for wm, dstl, scale in ((w["wq"], QT, 0.125), (w["wk"], KT, None)):
                            wch = []
                            for dc in range(DC):
                                wt = wp.tile([P, H * DK], F32R, tag="w", name="w")
                                nc.sync.dma_start(wt[:], wm[ds(P * dc, P), :])
                                wch.append(wt)
                            for tt in range(NTT):
                                pss = [qkps.tile([P, 512], F32, tag="qk", bufs=8, name="qkps")
                                       for _ in range(FC)]
                                for dc in range(DC):
                                    rhs = xcols(dc, glob, p, 512 * tt, 512)
                                    for fc in range(FC):
                                        nc.tensor.matmul(
                                            pss[fc][:], wch[dc][:, ds(P * fc, P)], rhs,
                                            start=(dc == 0), stop=(dc == DC - 1))
                                for fc in range(FC):
                                    dsl = dstl[fc][:, ds(512 * tt, 512)]
                                    nc.scalar.activation(dsl, pss[fc][:], AF.Copy,
                                                         scale=scale if scale else 1.0)

                    # wv chunks (used per-group below)
                    wvch = []
                    for dc in range(DC):
                        wt = wp.tile([P, H * DV], F32R, tag="w", name="w")
                        nc.sync.dma_start(wt[:], w["wv"][ds(P * dc, P), :])
                        wvch.append(wt)

                    # --- attention inner (per group of 128 tokens)
                    with (
                        tc.tile_pool(name="v_ps", bufs=1, space="PSUM") as vps_p,
                        tc.tile_pool(name="att_ps", bufs=1, space="PSUM") as aps,
                    ):
                        for g in range(NG):
                            vps = vps_p.tile([P, 2, 512], F32, tag="v", bufs=1)
                            if glob:
                                # stationary APs must be single-free-dim: stage the
                                # group's (strided) X columns contiguously first
                                xg_st = sp.tile([P, DC, P], F32R, tag="xgst", bufs=1)
                                for dc in range(DC):
                                    nc.vector.tensor_copy(
                                        gv(xg_st[:, dc, :], glob),
                                        xcols(dc, glob, p, P * g, P))
                            for dc in range(DC):
                                xg = xg_st[:, dc, :] if glob else xcols(dc, glob, p, P * g, P)
                                for hf in range(2):
                                    nc.tensor.matmul(
                                        vps[:, hf, :], xg, wvch[dc][:, ds(512 * hf, 512)],
                                        start=(dc == 0), stop=(dc == DC - 1))
                            v_t = sp.tile([P, H * DV], BF16, tag="v", bufs=2)
                            nc.scalar.activation(
                                v_t[:].rearrange("p (a b) -> p a b", a=2), vps[:], AF.Copy)
                            for h in range(H):
                                fc, hi = divmod(h, 2)
                                s_ps = aps.tile([P, P], F32, tag="s", bufs=2)
                                nc.tensor.matmul(
                                    s_ps[:],
                                    QT[fc][64 * hi:64 * hi + 64, ds(P * g, P)],
                                    KT[fc][64 * hi:64 * hi + 64, ds(P * g, P)],
                                    start=True, stop=False)
                                nc.tensor.matmul(s_ps[:], mq_t[:], mk_t[:],
                                                 start=False, stop=True)
                                pexp = sp.tile([P, P], F32, tag="pexp", bufs=2)
                                ssum = sp.tile([P, 1], F32, tag="ssum", bufs=6)
                                nc.scalar.activation(pexp[:], s_ps[:], AF.Exp,
                                                     accum_out=ssum[:])
                                srec = sp.tile([P, 1], F32, tag="srec", bufs=6)
                                nc.vector.reciprocal(srec[:], ssum[:])
                                pnorm = sp.tile([P, P], BF16, tag="pnorm", bufs=3)
                                nc.vector.tensor_scalar_mul(pnorm[:], pexp[:], srec[:])
                                pt_ps = aps.tile([P, P], BF16, tag="pt", bufs=2)
                                nc.tensor.transpose(pt_ps[:], pnorm[:], idbf_t[:])
                                ptn = sp.tile([P, P], BF16, tag="ptn", bufs=3)
                                nc.vector.scalar_tensor_tensor(
                                    ptn[:], pt_ps[:], 1.0, idbf_t[:],
                                    op0=ALU.mult, op1=ALU.subtract)
                                o_ps = aps.tile([64, P], F32, tag="o", bufs=2)
                                nc.tensor.matmul(o_ps[:], v_t[:, ds(64 * h, 64)], ptn[:],
                                                 start=True, stop=True)
                                nc.scalar.activation(
                                    attT[fc][64 * hi:64 * hi + 64, ds(P * g, P)],
                                    o_ps[:], AF.Copy)

                    # --- proj + residual -> Z (compact grouped)
                    pwch = []
                    for kc in range(DC):
                        wt = wp.tile([P, D], F32R, tag="w", name="w")
                        nc.sync.dma_start(wt[:], w["pw"][ds(P * kc, P), :])
                        pwch.append(wt)
                    with tc.tile_pool(name="pj_ps", bufs=1, space="PSUM") as pjps:
                        for tt in range(NTT):
                            pss = [pjps.tile([P, 512], F32, tag="pj", bufs=8, name="pjps")
                                   for _ in range(DC)]
                            for kc in range(DC):
                                rhs = attT[kc][:, ds(512 * tt, 512)]
                                for oc in range(DC):
                                    nc.tensor.matmul(
                                        pss[oc][:], pwch[kc][:, ds(P * oc, P)], rhs,
                                        start=(kc == 0), stop=(kc == DC - 1))
                            for oc in range(DC):
                                xd = xcols(oc, glob, p, 512 * tt, 512)
                                nc.vector.scalar_tensor_tensor(
                                    xd, gv(pss[oc][:], glob),
                                    vt["pb"][:, oc:oc + 1], xd,
                                    op0=ALU.add, op1=ALU.add)

                    # --- LN -> X
                    with (
                        tc.tile_pool(name="ln_w", bufs=1) as lnp,
                        tc.tile_pool(name="ln_ps", bufs=2, space="PSUM") as lnps,
                    ):
                        layer_norm_writeback(vt["g"], vt["b"], glob, p, lnp, lnps)

        # ---- FFN stage
        def ffn_stage(pfx):
            w = W[pfx]
            vt = VT[pfx]
            with tc.tile_pool(name=f"{pfx}_buf", bufs=1) as bp:
                for p in range(NPASS):
                    Yacc = [bp.tile([P, TP], F32, tag=f"y{oc}", name=f"y{oc}") for oc in range(DC)]
                    with (
                        tc.tile_pool(name="h_sb", bufs=2) as hp,
                        tc.tile_pool(name="h_ps", bufs=1, space="PSUM") as hps,
                    ):
                        for d in range(NDIC):
                            w1c = []
                            for dc in range(DC):
                                wt = wp.tile([P, 512], F32R, tag="w1s", name="w1s", bufs=8)
                                nc.sync.dma_start(
                                    wt[:], w["w1"][ds(P * dc, P), ds(512 * d, 512)])
                                w1c.append(wt)
                            w2c = []
                            for kc in range(4):
                                wt = wp.tile([P, D], F32R, tag="w", name="w")
                                nc.sync.dma_start(
                                    wt[:], w["w2"][ds(512 * d + P * kc, P), :])
                                w2c.append(wt)
                            Ht = [hp.tile([P, TP], F32R, tag=f"h{r}", name=f"h{r}") for r in range(4)]
                            for r in range(4):
                                for tt in range(NTT):
                                    ps = hps.tile([P, 512], F32, tag="h1", bufs=4)
                                    for dc in range(DC):
                                        nc.tensor.matmul(
                                            ps[:], w1c[dc][:, ds(P * r, P)],
                                            xcols(dc, False, p, 512 * tt, 512),
                                            start=(dc == 0), stop=(dc == DC - 1))
                                    nc.scalar.activation(
                                        Ht[r][:, ds(512 * tt, 512)], ps[:], AF.Relu,
                                        bias=vt["b1"][:, 4 * d + r:4 * d + r + 1])
                            for tt in range(NTT):
                                for oc in range(DC):
                                    ps2 = hps.tile([P, 512], F32, tag="w2ps", bufs=4)
                                    for kc in range(4):
                                        nc.tensor.matmul(
                                            ps2[:], w2c[kc][:, ds(P * oc, P)],
                                            Ht[kc][:, ds(512 * tt, 512)],
                                            start=(kc == 0), stop=(kc == 3))
                                    ysl = Yacc[oc][:, ds(512 * tt, 512)]
                                    if d == 0:
                                        nc.vector.tensor_copy(ysl, ps2[:])
                                    else:
                                        nc.vector.tensor_tensor(
                                            out=ysl, in0=ysl, in1=ps2[:], op=ALU.add)
                    # Z = Yacc + b2 + X ; LN -> X
                    with (
                        tc.tile_pool(name="ln_w", bufs=1) as lnp,
                        tc.tile_pool(name="ln_ps", bufs=2, space="PSUM") as lnps,
                    ):
                        for oc in range(DC):
                            for tt in range(NTT):
                                xd = xcols(oc, False, p, 512 * tt, 512)
                                nc.vector.scalar_tensor_tensor(
                                    xd, Yacc[oc][:, ds(512 * tt, 512)],
                                    vt["b2"][:, oc:oc + 1], xd,
                                    op0=ALU.add, op1=ALU.add)
                        layer_norm_writeback(vt["g"], vt["b"], False, p, lnp, lnps)

        attn_stage("la", glob=False)
        ffn_stage("lf")
        attn_stage("sa", glob=True)
        ffn_stage("pf")

        # ---- output transpose: X^T -> y [T, D]
        with (
            tc.tile_pool(name="oo_sb", bufs=3) as oop,
            tc.tile_pool(name="oo_ps", bufs=4, space="PSUM") as oops,
        ):
            for tg in range(T // P):
                nat = oop.tile([P, D], F32, tag="nat")
                for dc in range(DC):
                    tp = oops.tile([P, P], F32R, tag="tp")
                    nc.tensor.transpose(tp[:], X[dc][:, ds(P * tg, P)], id32r_t[:])
                    nc.scalar.activation(nat[:, ds(P * dc, P)], tp[:], AF.Copy)
                nc.sync.dma_start(y_out[ds(P * tg, P), :], nat[:])


# ------------------------------------------------------------------ host side

def _host_consts():
    r = MASK_C
    nloc = NL // 8 + 1  # 5
    mq_l = np.zeros((nloc, P), np.float32)
    mk_l = np.zeros((nloc, P), np.float32)
    for blk in range(P // NL):
        mq_l[blk, blk * NL:(blk + 1) * NL] = r
        mk_l[blk, blk * NL:(blk + 1) * NL] = r
    mq_l[-1, :] = r
    mk_l[-1, :] = -r
    mq_g = np.zeros((SPG + 1, P), np.float32)
    mk_g = np.zeros((SPG + 1, P), np.float32)
    for blk in range(SPG):
        mq_g[blk, blk * GSEQ:(blk + 1) * GSEQ] = r
        mk_g[blk, blk * GSEQ:(blk + 1) * GSEQ] = r
    mq_g[-1, :] = r
    mk_g[-1, :] = -r
    bf = ml_dtypes.bfloat16
    return dict(
        id32=np.eye(P, dtype=np.float32),
        id32r=np.eye(P, dtype=np.float32),
        idbf=np.eye(P, dtype=bf),
        mq_l=mq_l.astype(bf), mk_l=mk_l.astype(bf),
        mq_g=mq_g.astype(bf), mk_g=mk_g.astype(bf),
        ones_col=np.ones((P, 1), np.float32),
        eps_col=np.full((P, 1), EPS, np.float32),
        invd_row=np.full((1, P), 1.0 / D, np.float32),
    )


@functools.lru_cache(maxsize=2)
def _get_nc(repeat=1):
    return _build_nc(repeat)


def _shared_inputs(inputs):
    sh = {}
    for pfx in ("la", "sa"):
        sh[f"{pfx}_wq"] = np.ascontiguousarray(
            inputs[f"{pfx}_wqs"].transpose(1, 0, 2).reshape(D, H * DK))
        sh[f"{pfx}_wk"] = np.ascontiguousarray(
            inputs[f"{pfx}_wks"].transpose(1, 0, 2).reshape(D, H * DK))
        sh[f"{pfx}_wv"] = np.ascontiguousarray(
            inputs[f"{pfx}_wvs"].transpose(1, 0, 2).reshape(D, H * DV))
        sh[f"{pfx}_pw"] = np.ascontiguousarray(inputs[f"{pfx}_pw"])
        sh[f"{pfx}_pb"] = np.ascontiguousarray(inputs[f"{pfx}_pb"])
        sh[f"{pfx}_g"] = np.ascontiguousarray(inputs[f"{pfx}_g"])
        sh[f"{pfx}_b"] = np.ascontiguousarray(inputs[f"{pfx}_b"])
    for pfx in ("lf", "pf"):
        for k in ("w1", "b1", "w2", "b2", "g", "b"):
            sh[f"{pfx}_{k}"] = np.ascontiguousarray(inputs[f"{pfx}_{k}"])
    sh.update(_host_consts())
    return sh


def kernel(**inputs):
    nc = _get_nc()
    sh = _shared_inputs(inputs)
    x = np.asarray(inputs["enc_input"], dtype=np.float32)
    in_maps = []
    for c in range(B):
        m = dict(sh)
        m["x"] = np.ascontiguousarray(x[c])
        in_maps.append(m)
    res = run_bass_kernel_spmd(nc, in_maps, core_ids=list(range(B)))
    return np.stack([res.results[c]["y"] for c in range(B)], axis=0).astype(np.float32)

